# revision 11
# baseline (speedup 1.0000x reference)
"""Trainium2 Bass kernel for KosmosTextAttention (B=2, S=2048, E=2048, H=32).

Sharding: launch 1 = tensor-parallel over heads (4 groups) x data-parallel
over batch (2) -> 8 cores; launch 2 = row-parallel LayerNorm + out-proj
(LN needs full E, which head-sharded cores don't hold).

Launch 1 per core (b, g):
  qT/kT = W{q,k}T.T @ hsT  (out [e'=512, s])   - scale folded into Wq/bq on host
  v     = hsT.T @ WvT      (out [s, e'=512])   - stored interleaved [64 v | 1 ones]
  per head: scoresT[t,s] = kT_h.T-style matmul (K=64, head pairs packed at
  base partitions 0/64); p = exp(scoresT + maskT) (mask-add on DVE fused with
  PSUM eviction, exp batched on ACT); ctxT_unnorm[d,s] & row-sums via ones-
  augmented V matmul (M=65) accumulated over t in PSUM.
Host: normalize by sums, transpose to [s, e], gather full ctx, reshard rows.
Launch 2 per core (512 rows): LN stats + normalize (gamma/beta folded into
W2 = gamma*Wo.T and b2 = beta@Wo.T+bo on host), PE-transpose, out = y@W2+b2.
"""

import numpy as np

import concourse.bass as bass
import concourse.mybir as mybir
import concourse.tile as tile
from concourse import bacc
from concourse.bass_utils import run_bass_kernel_spmd
from concourse.masks import make_identity

B, S, E, H = 2, 2048, 2048, 32
D = 64
G = 4            # head groups
HG = H // G      # 8 heads per group
ES = E // G      # 512 channels per group
SCALE = D ** -0.5
LN_EPS = 1e-5
P = 128
FD = 512         # matmul free dim / s-block
NK = E // P      # 16 contraction tiles
NT = S // P      # 16 t tiles
NSB = S // FD    # 4 s blocks
NM = ES // P     # 4 output-channel tiles per group
CW = D + 1       # 65: v columns per head incl. ones column
EXPB = 4         # t-tiles batched per ACT exp op
f32 = mybir.dt.float32
AF = mybir.ActivationFunctionType
ALU = mybir.AluOpType


def build_launch1():
    nc = bacc.Bacc(None, target_bir_lowering=False)
    hsT = nc.declare_dram_parameter("hsT", [E, S], f32, isOutput=False)
    wqT = nc.declare_dram_parameter("wqT", [E, ES], f32, isOutput=False)
    wkT = nc.declare_dram_parameter("wkT", [E, ES], f32, isOutput=False)
    wvT = nc.declare_dram_parameter("wvT", [E, ES], f32, isOutput=False)
    bqkv = nc.declare_dram_parameter("bqkv", [3, ES], f32, isOutput=False)
    maskT = nc.declare_dram_parameter("maskT", [S, S], f32, isOutput=False)
    ctxT = nc.declare_dram_parameter("ctxT", [HG * CW, S], f32, isOutput=True)

    hsT_r = hsT.rearrange("(ko p) s -> p ko s", p=P)
    maskT_r = maskT.rearrange("(to p) s -> p to s", p=P)

    with tile.TileContext(nc) as tc:
        with tc.tile_pool(name="dram", bufs=1, space="DRAM") as dram:
            qT_d = dram.tile([ES, S], f32)
            kT_d = dram.tile([ES, S], f32)
            v_d = dram.tile([S, HG * CW], f32)

            # ---------------- phase 1: projections ----------------
            with (
                tc.tile_pool(name="hs_pool", bufs=1) as hs_pool,
                tc.tile_pool(name="wp", bufs=2) as wp,
                tc.tile_pool(name="bias_pool", bufs=1) as bias_pool,
                tc.tile_pool(name="stage", bufs=2) as stage_pool,
                tc.tile_pool(name="ppsum", bufs=4, space="PSUM") as ppsum,
            ):
                hs_sb = hs_pool.tile([P, NK, S], f32)
                for c in range(8):  # split the 16.8MB load across DMA queues
                    nc.sync.dma_start(
                        hs_sb[:, 2 * c : 2 * c + 2, :], hsT_r[:, 2 * c : 2 * c + 2, :]
                    )
                bias_sb = bias_pool.tile([P, 3, NM], f32)
                nc.sync.dma_start(
                    bias_sb, bqkv.rearrange("w (mo p) -> p w mo", p=P)
                )
                bv_rep = bias_pool.tile([P, ES], f32)
                nc.sync.dma_start(
                    bv_rep, bqkv[2:3, :].to_broadcast([P, ES])
                )

                # q and k: out tiles [e' 128, s 512]
                for w_idx, w_ap, out_d in ((0, wqT, qT_d), (1, wkT, kT_d)):
                    w_r = w_ap.rearrange("(ko p) m -> p ko m", p=P)
                    for m in range(NM):
                        w_sb = wp.tile([P, NK, P], f32, tag="wqk")
                        nc.sync.dma_start(w_sb, w_r[:, :, m * P : (m + 1) * P])
                        for sb in range(NSB):
                            ps = ppsum.tile([P, FD], f32, tag="proj")
                            for k in range(NK):
                                nc.tensor.matmul(
                                    ps,
                                    lhsT=w_sb[:, k, :],
                                    rhs=hs_sb[:, k, sb * FD : (sb + 1) * FD],
                                    start=(k == 0),
                                    stop=(k == NK - 1),
                                )
                            st = stage_pool.tile([P, FD], f32, tag="qk_st")
                            nc.vector.tensor_scalar(
                                out=st, in0=ps,
                                scalar1=bias_sb[:, w_idx, m : m + 1], scalar2=None,
                                op0=ALU.add,
                            )
                            nc.sync.dma_start(
                                out_d[m * P : (m + 1) * P, sb * FD : (sb + 1) * FD], st
                            )

                # v: out tiles [s 128, e' 512], interleaved with ones columns
                wv_r = wvT.rearrange("(ko p) m -> p ko m", p=P)
                wv_sb = wp.tile([P, NK, ES], f32, tag="wv", bufs=1)
                for c in range(4):
                    nc.sync.dma_start(
                        wv_sb[:, 4 * c : 4 * c + 4, :], wv_r[:, 4 * c : 4 * c + 4, :]
                    )
                for so in range(NT):
                    ps = ppsum.tile([P, ES], f32, tag="proj")
                    for k in range(NK):
                        nc.tensor.matmul(
                            ps,
                            lhsT=hs_sb[:, k, so * P : (so + 1) * P],
                            rhs=wv_sb[:, k, :],
                            start=(k == 0),
                            stop=(k == NK - 1),
                        )
                    vst = stage_pool.tile([P, HG * CW], f32, tag="v_st")
                    for h in range(HG):
                        nc.any.tensor_add(
                            out=vst[:, h * CW : h * CW + D],
                            in0=ps[:, h * D : (h + 1) * D],
                            in1=bv_rep[:, h * D : (h + 1) * D],
                        )
                        nc.any.memset(vst[:, h * CW + D : h * CW + D + 1], 1.0)
                    nc.sync.dma_start(v_d[so * P : (so + 1) * P, :], vst)

            # ---------------- phase 2: attention ----------------
            with (
                tc.tile_pool(name="qkv_res", bufs=1) as qkv_res,
                tc.tile_pool(name="mask_pool", bufs=1) as mask_pool,
                tc.tile_pool(name="p_pool", bufs=2) as p_pool,
                tc.tile_pool(name="cstage", bufs=3) as cstage,
                tc.tile_pool(name="sc_psum", bufs=4, space="PSUM") as sc_psum,
                tc.tile_pool(name="pv_psum", bufs=2, space="PSUM") as pv_psum,
            ):
                qT_sb = qkv_res.tile([P, NM, S], f32)
                kT_sb = qkv_res.tile([P, NM, S], f32)
                v_sb = qkv_res.tile([P, NT, HG * CW], f32)
                qr = qT_d.rearrange("(mo p) s -> p mo s", p=P)
                kr = kT_d.rearrange("(mo p) s -> p mo s", p=P)
                vr = v_d.rearrange("(so p) c -> p so c", p=P)
                for c in range(NM):
                    nc.sync.dma_start(qT_sb[:, c : c + 1, :], qr[:, c : c + 1, :])
                    nc.sync.dma_start(kT_sb[:, c : c + 1, :], kr[:, c : c + 1, :])
                for c in range(4):
                    nc.sync.dma_start(
                        v_sb[:, 4 * c : 4 * c + 4, :], vr[:, 4 * c : 4 * c + 4, :]
                    )

                for sb in range(NSB):
                    mk_sb = mask_pool.tile([P, NT, FD], f32, tag="mk")
                    for c in range(4):
                        nc.sync.dma_start(
                            mk_sb[:, 4 * c : 4 * c + 4, :],
                            maskT_r[:, 4 * c : 4 * c + 4, sb * FD : (sb + 1) * FD],
                        )
                    for j in range(HG // 2):  # head pairs
                        pv_ps = [
                            pv_psum.tile([CW, FD], f32, name=f"pv{i}", tag=f"pv{i}")
                            for i in range(2)
                        ]
                        for tb in range(NT // EXPB):  # exp batches
                            pt = [
                                p_pool.tile([P, EXPB, FD], f32, name=f"pt{i}",
                                            tag=f"pt{i}")
                                for i in range(2)
                            ]
                            for te in range(EXPB):
                                t = tb * EXPB + te
                                for i in range(2):
                                    h = 2 * j + i
                                    lo = D * (h % 2)
                                    sc = sc_psum.tile([P, FD], f32, tag="sc")
                                    nc.tensor.matmul(
                                        sc,
                                        lhsT=kT_sb[lo : lo + D, j, t * P : (t + 1) * P],
                                        rhs=qT_sb[lo : lo + D, j,
                                                  sb * FD : (sb + 1) * FD],
                                        start=True,
                                        stop=True,
                                    )
                                    nc.vector.tensor_tensor(
                                        pt[i][:, te, :], sc, mk_sb[:, t, :], ALU.add
                                    )
                            for i in range(2):
                                nc.scalar.activation(pt[i], pt[i], AF.Exp)
                            for te in range(EXPB):
                                t = tb * EXPB + te
                                for i in range(2):
                                    h = 2 * j + i
                                    nc.tensor.matmul(
                                        pv_ps[i],
                                        lhsT=v_sb[:, t, h * CW : (h + 1) * CW],
                                        rhs=pt[i][:, te, :],
                                        start=(t == 0),
                                        stop=(t == NT - 1),
                                    )
                        for i in range(2):
                            h = 2 * j + i
                            cst = cstage.tile([CW, FD], f32, tag="cst")
                            nc.any.tensor_copy(out=cst, in_=pv_ps[i])
                            nc.sync.dma_start(
                                ctxT[h * CW : (h + 1) * CW, sb * FD : (sb + 1) * FD],
                                cst,
                            )
    nc.compile()
    return nc


def build_launch2():
    RPC = B * S // 8  # 512 rows per core
    nc = bacc.Bacc(None, target_bir_lowering=False)
    xc = nc.declare_dram_parameter("xc", [RPC, E], f32, isOutput=False)
    w2 = nc.declare_dram_parameter("w2", [E, E], f32, isOutput=False)
    b2 = nc.declare_dram_parameter("b2", [1, E], f32, isOutput=False)
    outr = nc.declare_dram_parameter("outr", [RPC, E], f32, isOutput=True)

    NMT = RPC // P  # 4 row tiles
    NNT = E // FD   # 4 out-column tiles
    w2_r = w2.rearrange("(ko p) e -> p ko e", p=P)
    xc_r = xc.rearrange("(mo p) e -> p mo e", p=P)

    with tile.TileContext(nc) as tc:
        with (
            tc.tile_pool(name="const2", bufs=1) as const2,
            tc.tile_pool(name="xp", bufs=2) as xp,
            tc.tile_pool(name="statp", bufs=4) as statp,
            tc.tile_pool(name="ytp", bufs=1) as ytp,
            tc.tile_pool(name="w2p", bufs=2) as w2p,
            tc.tile_pool(name="ostage", bufs=3) as ostage,
            tc.tile_pool(name="tpsum", bufs=2, space="PSUM") as tpsum,
            tc.tile_pool(name="opsum", bufs=3, space="PSUM") as opsum,
        ):
            ident = const2.tile([P, P], f32)
            make_identity(nc, ident)
            b2_rep = const2.tile([P, E], f32)
            nc.sync.dma_start(b2_rep, b2[0:1, :].to_broadcast([P, E]))
            eps_sb = const2.tile([P, 1], f32)
            nc.any.memset(eps_sb, LN_EPS)
            yT = ytp.tile([P, NK, RPC], f32)

            for mt in range(NMT):
                x = xp.tile([P, E], f32, tag="x")
                nc.sync.dma_start(x, xc_r[:, mt, :])
                sq = xp.tile([P, E], f32, tag="sq")
                nc.scalar.activation(sq, x, AF.Square)
                s1 = statp.tile([P, 1], f32, tag="s1")
                s2 = statp.tile([P, 1], f32, tag="s2")
                nc.vector.reduce_sum(s1, x, axis=mybir.AxisListType.X)
                nc.vector.reduce_sum(s2, sq, axis=mybir.AxisListType.X)
                mu = statp.tile([P, 1], f32, tag="mu")
                nc.vector.tensor_scalar_mul(mu, s1, 1.0 / E)
                var = statp.tile([P, 1], f32, tag="var")
                # var = s2/E - mu^2
                musq = statp.tile([P, 1], f32, tag="musq")
                nc.vector.tensor_tensor(musq, mu, mu, ALU.mult)
                nc.vector.tensor_scalar(
                    out=var, in0=s2, scalar1=1.0 / E, scalar2=None, op0=ALU.mult
                )
                nc.vector.tensor_tensor(var, var, musq, ALU.subtract)
                sd = statp.tile([P, 1], f32, tag="sd")
                nc.scalar.activation(sd, var, AF.Sqrt, bias=eps_sb)
                r = statp.tile([P, 1], f32, tag="r")
                nc.vector.reciprocal(r, sd)
                nmr = statp.tile([P, 1], f32, tag="nmr")
                nc.vector.tensor_tensor(nmr, mu, r, ALU.mult)
                nc.vector.tensor_scalar_mul(nmr, nmr, -1.0)
                y = xp.tile([P, E], f32, tag="y")
                nc.vector.tensor_scalar(
                    out=y, in0=x, scalar1=r, scalar2=nmr, op0=ALU.mult, op1=ALU.add
                )
                for k in range(NK):
                    tp = tpsum.tile([P, P], f32, tag="tp")
                    nc.tensor.transpose(tp, y[:, k * P : (k + 1) * P], ident)
                    nc.any.tensor_copy(
                        out=yT[:, k, mt * P : (mt + 1) * P], in_=tp
                    )

            for nt in range(NNT):
                w_sb = w2p.tile([P, NK, FD], f32, tag="w2")
                for c in range(4):
                    nc.sync.dma_start(
                        w_sb[:, 4 * c : 4 * c + 4, :],
                        w2_r[:, 4 * c : 4 * c + 4, nt * FD : (nt + 1) * FD],
                    )
                for mt in range(NMT):
                    ps = opsum.tile([P, FD], f32, tag="ops")
                    for k in range(NK):
                        nc.tensor.matmul(
                            ps,
                            lhsT=yT[:, k, mt * P : (mt + 1) * P],
                            rhs=w_sb[:, k, :],
                            start=(k == 0),
                            stop=(k == NK - 1),
                        )
                    ost = ostage.tile([P, FD], f32, tag="ost")
                    nc.any.tensor_add(
                        out=ost, in0=ps, in1=b2_rep[:, nt * FD : (nt + 1) * FD]
                    )
                    nc.sync.dma_start(
                        outr.rearrange("(mo p) e -> p mo e", p=P)[
                            :, mt, nt * FD : (nt + 1) * FD
                        ],
                        ost,
                    )
    nc.compile()
    return nc


def _prep_launch1_inputs(hidden_states, attention_mask, Wq, bq, Wk, bk, Wv, bv):
    hsT = [np.ascontiguousarray(hidden_states[b].T) for b in range(B)]
    maskT = [np.ascontiguousarray(attention_mask[b, 0].T) for b in range(B)]
    in_maps = []
    for c in range(8):
        b, g = c // G, c % G
        sl = slice(g * ES, (g + 1) * ES)
        in_maps.append({
            "hsT": hsT[b],
            "wqT": np.ascontiguousarray(Wq[sl, :].T * SCALE),
            "wkT": np.ascontiguousarray(Wk[sl, :].T),
            "wvT": np.ascontiguousarray(Wv[sl, :].T),
            "bqkv": np.ascontiguousarray(
                np.stack([bq[sl] * SCALE, bk[sl], bv[sl]])
            ),
            "maskT": maskT[b],
        })
    return in_maps


def _assemble_ctx(results1):
    """results1[c]["ctxT"] [520, 2048] -> full ctx [B*S, E]."""
    ctx = np.empty((B * S, E), dtype=np.float32)
    for c in range(8):
        b, g = c // G, c % G
        arr = results1[c]["ctxT"].reshape(HG, CW, S)
        normed = arr[:, :D, :] / arr[:, D : D + 1, :]   # [HG, D, S]
        # -> [S, HG*D]
        ctx[b * S : (b + 1) * S, g * ES : (g + 1) * ES] = (
            normed.transpose(2, 0, 1).reshape(S, ES)
        )
    return ctx


def run_pipeline(inputs, trace=False):
    hidden_states = np.asarray(inputs["hidden_states"], dtype=np.float32)
    attention_mask = np.asarray(inputs["attention_mask"], dtype=np.float32)
    Wq = np.asarray(inputs["Wq"], dtype=np.float32)
    Wk = np.asarray(inputs["Wk"], dtype=np.float32)
    Wv = np.asarray(inputs["Wv"], dtype=np.float32)
    Wo = np.asarray(inputs["Wo"], dtype=np.float32)
    bq = np.asarray(inputs["bq"], dtype=np.float32)
    bk = np.asarray(inputs["bk"], dtype=np.float32)
    bv = np.asarray(inputs["bv"], dtype=np.float32)
    bo = np.asarray(inputs["bo"], dtype=np.float32)
    ln_gamma = np.asarray(inputs["ln_gamma"], dtype=np.float32)
    ln_beta = np.asarray(inputs["ln_beta"], dtype=np.float32)

    core_ids = list(range(8))
    nc1 = build_launch1()
    in_maps1 = _prep_launch1_inputs(
        hidden_states, attention_mask, Wq, bq, Wk, bk, Wv, bv
    )
    res1 = run_bass_kernel_spmd(nc1, in_maps1, core_ids, trace=trace)
    ctx = _assemble_ctx(res1.results)

    # launch 2: fold gamma/beta into out-proj
    w2 = np.ascontiguousarray(ln_gamma[:, None] * Wo.T)
    b2 = np.ascontiguousarray((ln_beta @ Wo.T + bo)[None, :])
    RPC = B * S // 8
    nc2 = build_launch2()
    in_maps2 = [
        {"xc": np.ascontiguousarray(ctx[c * RPC : (c + 1) * RPC]), "w2": w2,
         "b2": b2}
        for c in range(8)
    ]
    res2 = run_bass_kernel_spmd(nc2, in_maps2, core_ids, trace=trace)
    out = np.concatenate([res2.results[c]["outr"] for c in range(8)], axis=0)
    out = out.reshape(B, S, E)
    ns = None
    if trace:
        parts = [r.exec_time_ns for r in (res1, res2)]
        if all(p is not None for p in parts):
            ns = sum(parts)
    return out, ns, (res1, res2)


def kernel(**inputs):
    out, _, _ = run_pipeline(inputs, trace=False)
    return out


# revision 13
# speedup vs baseline: 2.6165x; 2.6165x over previous
"""Trainium2 Bass kernel for KosmosTextAttention (B=2, S=2048, E=2048, H=32).

Sharding: launch 1 = tensor-parallel over heads (4 groups) x data-parallel
over batch (2) -> 8 cores; launch 2 = row-parallel LayerNorm + out-proj
(LN needs full E, which head-sharded cores don't hold).

Launch 1 per core (b, g):
  qT/kT = W{q,k}T.T @ hsT  (out [e'=512, s])   - scale folded into Wq/bq on host
  v     = hsT.T @ WvT      (out [s, e'=512])   - stored interleaved [64 v | 1 ones]
  per head: scoresT[t,s] = kT_h.T-style matmul (K=64, head pairs packed at
  base partitions 0/64); p = exp(scoresT + maskT) (mask-add on DVE fused with
  PSUM eviction, exp batched on ACT); ctxT_unnorm[d,s] & row-sums via ones-
  augmented V matmul (M=65) accumulated over t in PSUM.
Host: normalize by sums, transpose to [s, e], gather full ctx, reshard rows.
Launch 2 per core (512 rows): LN stats + normalize (gamma/beta folded into
W2 = gamma*Wo.T and b2 = beta@Wo.T+bo on host), PE-transpose, out = y@W2+b2.
"""

import numpy as np

import concourse.bass as bass
import concourse.mybir as mybir
import concourse.tile as tile
from concourse import bacc
from concourse.bass_utils import run_bass_kernel_spmd
from concourse.masks import make_identity

B, S, E, H = 2, 2048, 2048, 32
D = 64
G = 4            # head groups
HG = H // G      # 8 heads per group
ES = E // G      # 512 channels per group
SCALE = D ** -0.5
LN_EPS = 1e-5
P = 128
FD = 512         # matmul free dim / s-block
NK = E // P      # 16 contraction tiles
NT = S // P      # 16 t tiles
NSB = S // FD    # 4 s blocks
NM = ES // P     # 4 output-channel tiles per group
CW = D + 1       # 65: v columns per head incl. ones column
EXPB = 4         # t-tiles batched per ACT exp op
f32 = mybir.dt.float32
f32r = mybir.dt.float32r
AF = mybir.ActivationFunctionType
ALU = mybir.AluOpType
USE_F32R = True  # stream fp32 matmuls in single-pass float32r mode (4x PE rate)


def _mm(nc, out, lhsT, rhs, **kw):
    if USE_F32R and lhsT.dtype == f32:
        lhsT = lhsT.bitcast(f32r)
        rhs = rhs.bitcast(f32r)
    nc.tensor.matmul(out, lhsT=lhsT, rhs=rhs, **kw)


def build_launch1():
    nc = bacc.Bacc(None, target_bir_lowering=False)
    hsT = nc.declare_dram_parameter("hsT", [E, S], f32, isOutput=False)
    wqT = nc.declare_dram_parameter("wqT", [E, ES], f32, isOutput=False)
    wkT = nc.declare_dram_parameter("wkT", [E, ES], f32, isOutput=False)
    wvT = nc.declare_dram_parameter("wvT", [E, ES], f32, isOutput=False)
    bqkv = nc.declare_dram_parameter("bqkv", [3, ES], f32, isOutput=False)
    maskT = nc.declare_dram_parameter("maskT", [S, S], f32, isOutput=False)
    ctxT = nc.declare_dram_parameter("ctxT", [HG * CW, S], f32, isOutput=True)

    hsT_r = hsT.rearrange("(ko p) s -> p ko s", p=P)
    maskT_r = maskT.rearrange("(to p) s -> p to s", p=P)

    with tile.TileContext(nc) as tc:
        with tc.tile_pool(name="dram", bufs=1, space="DRAM") as dram:
            qT_d = dram.tile([ES, S], f32)
            kT_d = dram.tile([ES, S], f32)
            v_d = dram.tile([S, HG * CW], f32)

            # ---------------- phase 1: projections ----------------
            with (
                tc.tile_pool(name="hs_pool", bufs=1) as hs_pool,
                tc.tile_pool(name="wp", bufs=2) as wp,
                tc.tile_pool(name="bias_pool", bufs=1) as bias_pool,
                tc.tile_pool(name="stage", bufs=2) as stage_pool,
                tc.tile_pool(name="ppsum", bufs=4, space="PSUM") as ppsum,
            ):
                hs_sb = hs_pool.tile([P, NK, S], f32)
                for c in range(8):  # split the 16.8MB load across DMA queues
                    nc.sync.dma_start(
                        hs_sb[:, 2 * c : 2 * c + 2, :], hsT_r[:, 2 * c : 2 * c + 2, :]
                    )
                bias_sb = bias_pool.tile([P, 3, NM], f32)
                nc.sync.dma_start(
                    bias_sb, bqkv.rearrange("w (mo p) -> p w mo", p=P)
                )
                bv_rep = bias_pool.tile([P, ES], f32)
                nc.sync.dma_start(
                    bv_rep, bqkv[2:3, :].to_broadcast([P, ES])
                )

                # q and k: out tiles [e' 128, s 512]
                for w_idx, w_ap, out_d in ((0, wqT, qT_d), (1, wkT, kT_d)):
                    w_r = w_ap.rearrange("(ko p) m -> p ko m", p=P)
                    for m in range(NM):
                        w_sb = wp.tile([P, NK, P], f32, tag="wqk")
                        nc.sync.dma_start(w_sb, w_r[:, :, m * P : (m + 1) * P])
                        for sb in range(NSB):
                            ps = ppsum.tile([P, FD], f32, tag="proj")
                            for k in range(NK):
                                _mm(
                                    nc, ps,
                                    lhsT=w_sb[:, k, :],
                                    rhs=hs_sb[:, k, sb * FD : (sb + 1) * FD],
                                    start=(k == 0),
                                    stop=(k == NK - 1),
                                )
                            st = stage_pool.tile([P, FD], f32, tag="qk_st")
                            nc.vector.tensor_scalar(
                                out=st, in0=ps,
                                scalar1=bias_sb[:, w_idx, m : m + 1], scalar2=None,
                                op0=ALU.add,
                            )
                            nc.sync.dma_start(
                                out_d[m * P : (m + 1) * P, sb * FD : (sb + 1) * FD], st
                            )

                # v: out tiles [s 128, e' 512], interleaved with ones columns
                wv_r = wvT.rearrange("(ko p) m -> p ko m", p=P)
                wv_sb = wp.tile([P, NK, ES], f32, tag="wv", bufs=1)
                for c in range(4):
                    nc.sync.dma_start(
                        wv_sb[:, 4 * c : 4 * c + 4, :], wv_r[:, 4 * c : 4 * c + 4, :]
                    )
                for so in range(NT):
                    ps = ppsum.tile([P, ES], f32, tag="proj")
                    for k in range(NK):
                        _mm(
                            nc, ps,
                            lhsT=hs_sb[:, k, so * P : (so + 1) * P],
                            rhs=wv_sb[:, k, :],
                            start=(k == 0),
                            stop=(k == NK - 1),
                        )
                    vst = stage_pool.tile([P, HG * CW], f32, tag="v_st")
                    for h in range(HG):
                        nc.any.tensor_add(
                            out=vst[:, h * CW : h * CW + D],
                            in0=ps[:, h * D : (h + 1) * D],
                            in1=bv_rep[:, h * D : (h + 1) * D],
                        )
                        nc.any.memset(vst[:, h * CW + D : h * CW + D + 1], 1.0)
                    nc.sync.dma_start(v_d[so * P : (so + 1) * P, :], vst)

            # ---------------- phase 2: attention ----------------
            with (
                tc.tile_pool(name="qkv_res", bufs=1) as qkv_res,
                tc.tile_pool(name="mask_pool", bufs=1) as mask_pool,
                tc.tile_pool(name="p_pool", bufs=2) as p_pool,
                tc.tile_pool(name="cstage", bufs=3) as cstage,
                tc.tile_pool(name="sc_psum", bufs=4, space="PSUM") as sc_psum,
                tc.tile_pool(name="pv_psum", bufs=2, space="PSUM") as pv_psum,
            ):
                qT_sb = qkv_res.tile([P, NM, S], f32)
                kT_sb = qkv_res.tile([P, NM, S], f32)
                v_sb = qkv_res.tile([P, NT, HG * CW], f32)
                qr = qT_d.rearrange("(mo p) s -> p mo s", p=P)
                kr = kT_d.rearrange("(mo p) s -> p mo s", p=P)
                vr = v_d.rearrange("(so p) c -> p so c", p=P)
                for c in range(NM):
                    nc.sync.dma_start(qT_sb[:, c : c + 1, :], qr[:, c : c + 1, :])
                    nc.sync.dma_start(kT_sb[:, c : c + 1, :], kr[:, c : c + 1, :])
                for c in range(4):
                    nc.sync.dma_start(
                        v_sb[:, 4 * c : 4 * c + 4, :], vr[:, 4 * c : 4 * c + 4, :]
                    )

                for sb in range(NSB):
                    mk_sb = mask_pool.tile([P, NT, FD], f32, tag="mk")
                    for c in range(4):
                        nc.sync.dma_start(
                            mk_sb[:, 4 * c : 4 * c + 4, :],
                            maskT_r[:, 4 * c : 4 * c + 4, sb * FD : (sb + 1) * FD],
                        )
                    for j in range(HG // 2):  # head pairs
                        pv_ps = [
                            pv_psum.tile([CW, FD], f32, name=f"pv{i}", tag=f"pv{i}")
                            for i in range(2)
                        ]
                        for tb in range(NT // EXPB):  # exp batches
                            pt = [
                                p_pool.tile([P, EXPB, FD], f32, name=f"pt{i}",
                                            tag=f"pt{i}")
                                for i in range(2)
                            ]
                            for te in range(EXPB):
                                t = tb * EXPB + te
                                for i in range(2):
                                    h = 2 * j + i
                                    lo = D * (h % 2)
                                    sc = sc_psum.tile([P, FD], f32, tag="sc")
                                    _mm(
                                        nc, sc,
                                        lhsT=kT_sb[lo : lo + D, j, t * P : (t + 1) * P],
                                        rhs=qT_sb[lo : lo + D, j,
                                                  sb * FD : (sb + 1) * FD],
                                        start=True,
                                        stop=True,
                                    )
                                    nc.vector.tensor_tensor(
                                        pt[i][:, te, :], sc, mk_sb[:, t, :], ALU.add
                                    )
                            for i in range(2):
                                nc.scalar.activation(pt[i], pt[i], AF.Exp)
                            for te in range(EXPB):
                                t = tb * EXPB + te
                                for i in range(2):
                                    h = 2 * j + i
                                    _mm(
                                        nc, pv_ps[i],
                                        lhsT=v_sb[:, t, h * CW : (h + 1) * CW],
                                        rhs=pt[i][:, te, :],
                                        start=(t == 0),
                                        stop=(t == NT - 1),
                                    )
                        for i in range(2):
                            h = 2 * j + i
                            cst = cstage.tile([CW, FD], f32, tag="cst")
                            nc.any.tensor_copy(out=cst, in_=pv_ps[i])
                            nc.sync.dma_start(
                                ctxT[h * CW : (h + 1) * CW, sb * FD : (sb + 1) * FD],
                                cst,
                            )
    nc.compile()
    return nc


def build_launch2():
    RPC = B * S // 8  # 512 rows per core
    nc = bacc.Bacc(None, target_bir_lowering=False)
    xc = nc.declare_dram_parameter("xc", [RPC, E], f32, isOutput=False)
    w2 = nc.declare_dram_parameter("w2", [E, E], f32, isOutput=False)
    b2 = nc.declare_dram_parameter("b2", [1, E], f32, isOutput=False)
    outr = nc.declare_dram_parameter("outr", [RPC, E], f32, isOutput=True)

    NMT = RPC // P  # 4 row tiles
    NNT = E // FD   # 4 out-column tiles
    w2_r = w2.rearrange("(ko p) e -> p ko e", p=P)
    xc_r = xc.rearrange("(mo p) e -> p mo e", p=P)

    with tile.TileContext(nc) as tc:
        with (
            tc.tile_pool(name="const2", bufs=1) as const2,
            tc.tile_pool(name="xp", bufs=2) as xp,
            tc.tile_pool(name="statp", bufs=4) as statp,
            tc.tile_pool(name="ytp", bufs=1) as ytp,
            tc.tile_pool(name="w2p", bufs=2) as w2p,
            tc.tile_pool(name="ostage", bufs=3) as ostage,
            tc.tile_pool(name="tpsum", bufs=2, space="PSUM") as tpsum,
            tc.tile_pool(name="opsum", bufs=3, space="PSUM") as opsum,
        ):
            ident = const2.tile([P, P], f32)
            make_identity(nc, ident)
            b2_rep = const2.tile([P, E], f32)
            nc.sync.dma_start(b2_rep, b2[0:1, :].to_broadcast([P, E]))
            eps_sb = const2.tile([P, 1], f32)
            nc.any.memset(eps_sb, LN_EPS)
            yT = ytp.tile([P, NK, RPC], f32)

            for mt in range(NMT):
                x = xp.tile([P, E], f32, tag="x")
                nc.sync.dma_start(x, xc_r[:, mt, :])
                sq = xp.tile([P, E], f32, tag="sq")
                nc.scalar.activation(sq, x, AF.Square)
                s1 = statp.tile([P, 1], f32, tag="s1")
                s2 = statp.tile([P, 1], f32, tag="s2")
                nc.vector.reduce_sum(s1, x, axis=mybir.AxisListType.X)
                nc.vector.reduce_sum(s2, sq, axis=mybir.AxisListType.X)
                mu = statp.tile([P, 1], f32, tag="mu")
                nc.vector.tensor_scalar_mul(mu, s1, 1.0 / E)
                var = statp.tile([P, 1], f32, tag="var")
                # var = s2/E - mu^2
                musq = statp.tile([P, 1], f32, tag="musq")
                nc.vector.tensor_tensor(musq, mu, mu, ALU.mult)
                nc.vector.tensor_scalar(
                    out=var, in0=s2, scalar1=1.0 / E, scalar2=None, op0=ALU.mult
                )
                nc.vector.tensor_tensor(var, var, musq, ALU.subtract)
                sd = statp.tile([P, 1], f32, tag="sd")
                nc.scalar.activation(sd, var, AF.Sqrt, bias=eps_sb)
                r = statp.tile([P, 1], f32, tag="r")
                nc.vector.reciprocal(r, sd)
                nmr = statp.tile([P, 1], f32, tag="nmr")
                nc.vector.tensor_tensor(nmr, mu, r, ALU.mult)
                nc.vector.tensor_scalar_mul(nmr, nmr, -1.0)
                y = xp.tile([P, E], f32, tag="y")
                nc.vector.tensor_scalar(
                    out=y, in0=x, scalar1=r, scalar2=nmr, op0=ALU.mult, op1=ALU.add
                )
                for k in range(NK):
                    tp = tpsum.tile([P, P], f32, tag="tp")
                    nc.tensor.transpose(tp, y[:, k * P : (k + 1) * P], ident)
                    nc.any.tensor_copy(
                        out=yT[:, k, mt * P : (mt + 1) * P], in_=tp
                    )

            for nt in range(NNT):
                w_sb = w2p.tile([P, NK, FD], f32, tag="w2")
                for c in range(4):
                    nc.sync.dma_start(
                        w_sb[:, 4 * c : 4 * c + 4, :],
                        w2_r[:, 4 * c : 4 * c + 4, nt * FD : (nt + 1) * FD],
                    )
                for mt in range(NMT):
                    ps = opsum.tile([P, FD], f32, tag="ops")
                    for k in range(NK):
                        _mm(
                            nc, ps,
                            lhsT=yT[:, k, mt * P : (mt + 1) * P],
                            rhs=w_sb[:, k, :],
                            start=(k == 0),
                            stop=(k == NK - 1),
                        )
                    ost = ostage.tile([P, FD], f32, tag="ost")
                    nc.any.tensor_add(
                        out=ost, in0=ps, in1=b2_rep[:, nt * FD : (nt + 1) * FD]
                    )
                    nc.sync.dma_start(
                        outr.rearrange("(mo p) e -> p mo e", p=P)[
                            :, mt, nt * FD : (nt + 1) * FD
                        ],
                        ost,
                    )
    nc.compile()
    return nc


def _prep_launch1_inputs(hidden_states, attention_mask, Wq, bq, Wk, bk, Wv, bv):
    hsT = [np.ascontiguousarray(hidden_states[b].T) for b in range(B)]
    maskT = [np.ascontiguousarray(attention_mask[b, 0].T) for b in range(B)]
    in_maps = []
    for c in range(8):
        b, g = c // G, c % G
        sl = slice(g * ES, (g + 1) * ES)
        in_maps.append({
            "hsT": hsT[b],
            "wqT": np.ascontiguousarray(Wq[sl, :].T * SCALE),
            "wkT": np.ascontiguousarray(Wk[sl, :].T),
            "wvT": np.ascontiguousarray(Wv[sl, :].T),
            "bqkv": np.ascontiguousarray(
                np.stack([bq[sl] * SCALE, bk[sl], bv[sl]])
            ),
            "maskT": maskT[b],
        })
    return in_maps


def _assemble_ctx(results1):
    """results1[c]["ctxT"] [520, 2048] -> full ctx [B*S, E]."""
    ctx = np.empty((B * S, E), dtype=np.float32)
    for c in range(8):
        b, g = c // G, c % G
        arr = results1[c]["ctxT"].reshape(HG, CW, S)
        normed = arr[:, :D, :] / arr[:, D : D + 1, :]   # [HG, D, S]
        # -> [S, HG*D]
        ctx[b * S : (b + 1) * S, g * ES : (g + 1) * ES] = (
            normed.transpose(2, 0, 1).reshape(S, ES)
        )
    return ctx


def run_pipeline(inputs, trace=False):
    hidden_states = np.asarray(inputs["hidden_states"], dtype=np.float32)
    attention_mask = np.asarray(inputs["attention_mask"], dtype=np.float32)
    Wq = np.asarray(inputs["Wq"], dtype=np.float32)
    Wk = np.asarray(inputs["Wk"], dtype=np.float32)
    Wv = np.asarray(inputs["Wv"], dtype=np.float32)
    Wo = np.asarray(inputs["Wo"], dtype=np.float32)
    bq = np.asarray(inputs["bq"], dtype=np.float32)
    bk = np.asarray(inputs["bk"], dtype=np.float32)
    bv = np.asarray(inputs["bv"], dtype=np.float32)
    bo = np.asarray(inputs["bo"], dtype=np.float32)
    ln_gamma = np.asarray(inputs["ln_gamma"], dtype=np.float32)
    ln_beta = np.asarray(inputs["ln_beta"], dtype=np.float32)

    core_ids = list(range(8))
    nc1 = build_launch1()
    in_maps1 = _prep_launch1_inputs(
        hidden_states, attention_mask, Wq, bq, Wk, bk, Wv, bv
    )
    res1 = run_bass_kernel_spmd(nc1, in_maps1, core_ids, trace=trace)
    ctx = _assemble_ctx(res1.results)

    # launch 2: fold gamma/beta into out-proj
    w2 = np.ascontiguousarray(ln_gamma[:, None] * Wo.T)
    b2 = np.ascontiguousarray((ln_beta @ Wo.T + bo)[None, :])
    RPC = B * S // 8
    nc2 = build_launch2()
    in_maps2 = [
        {"xc": np.ascontiguousarray(ctx[c * RPC : (c + 1) * RPC]), "w2": w2,
         "b2": b2}
        for c in range(8)
    ]
    res2 = run_bass_kernel_spmd(nc2, in_maps2, core_ids, trace=trace)
    out = np.concatenate([res2.results[c]["outr"] for c in range(8)], axis=0)
    out = out.reshape(B, S, E)
    ns = None
    if trace:
        parts = [r.exec_time_ns for r in (res1, res2)]
        if all(p is not None for p in parts):
            ns = sum(parts)
    return out, ns, (res1, res2)


def kernel(**inputs):
    out, _, _ = run_pipeline(inputs, trace=False)
    return out


# revision 15
# speedup vs baseline: 2.6422x; 1.0098x over previous
"""Trainium2 Bass kernel for KosmosTextAttention (B=2, S=2048, E=2048, H=32).

Sharding: launch 1 = tensor-parallel over heads (4 groups) x data-parallel
over batch (2) -> 8 cores; launch 2 = row-parallel LayerNorm + out-proj
(LN needs full E, which head-sharded cores don't hold).

Launch 1 per core (b, g):
  qT/kT = W{q,k}T.T @ hsT  (out [e'=512, s])   - scale folded into Wq/bq on host
  v     = hsT.T @ WvT      (out [s, e'=512])   - stored interleaved [64 v | 1 ones]
  per head: scoresT[t,s] = kT_h.T-style matmul (K=64, head pairs packed at
  base partitions 0/64); p = exp(scoresT + maskT) (mask-add on DVE fused with
  PSUM eviction, exp batched on ACT); ctxT_unnorm[d,s] & row-sums via ones-
  augmented V matmul (M=65) accumulated over t in PSUM.
Host: normalize by sums, transpose to [s, e], gather full ctx, reshard rows.
Launch 2 per core (512 rows): LN stats + normalize (gamma/beta folded into
W2 = gamma*Wo.T and b2 = beta@Wo.T+bo on host), PE-transpose, out = y@W2+b2.
"""

import numpy as np

import concourse.bass as bass
import concourse.mybir as mybir
import concourse.tile as tile
from concourse import bacc
from concourse.bass_utils import run_bass_kernel_spmd
from concourse.masks import make_identity

B, S, E, H = 2, 2048, 2048, 32
D = 64
G = 4            # head groups
HG = H // G      # 8 heads per group
ES = E // G      # 512 channels per group
SCALE = D ** -0.5
LN_EPS = 1e-5
P = 128
FD = 512         # matmul free dim / s-block
NK = E // P      # 16 contraction tiles
NT = S // P      # 16 t tiles
NSB = S // FD    # 4 s blocks
NM = ES // P     # 4 output-channel tiles per group
CW = D + 1       # 65: v columns per head incl. ones column
EXPB = 4         # t-tiles batched per ACT exp op
f32 = mybir.dt.float32
f32r = mybir.dt.float32r
AF = mybir.ActivationFunctionType
ALU = mybir.AluOpType
USE_F32R = True  # stream fp32 matmuls in single-pass float32r mode (4x PE rate)


def _mm(nc, out, lhsT, rhs, **kw):
    if USE_F32R and lhsT.dtype == f32:
        lhsT = lhsT.bitcast(f32r)
        rhs = rhs.bitcast(f32r)
    nc.tensor.matmul(out, lhsT=lhsT, rhs=rhs, **kw)


def build_launch1():
    nc = bacc.Bacc(None, target_bir_lowering=False)
    hsT = nc.declare_dram_parameter("hsT", [E, S], f32, isOutput=False)
    wqT = nc.declare_dram_parameter("wqT", [E, ES], f32, isOutput=False)
    wkT = nc.declare_dram_parameter("wkT", [E, ES], f32, isOutput=False)
    wvT = nc.declare_dram_parameter("wvT", [E, ES], f32, isOutput=False)
    bqkv = nc.declare_dram_parameter("bqkv", [3, ES], f32, isOutput=False)
    maskT = nc.declare_dram_parameter("maskT", [S, S], f32, isOutput=False)
    ctxT = nc.declare_dram_parameter("ctxT", [HG * CW, S], f32, isOutput=True)

    hsT_r = hsT.rearrange("(ko p) s -> p ko s", p=P)
    maskT_r = maskT.rearrange("(to p) s -> p to s", p=P)

    with tile.TileContext(nc) as tc:
        with tc.tile_pool(name="dram", bufs=1, space="DRAM") as dram:
            qT_d = dram.tile([ES, S], f32r)
            kT_d = dram.tile([ES, S], f32r)
            v_d = dram.tile([S, HG * CW], f32r)

            # ---------------- phase 1: projections ----------------
            with (
                tc.tile_pool(name="hs_pool", bufs=1) as hs_pool,
                tc.tile_pool(name="wp", bufs=2) as wp,
                tc.tile_pool(name="bias_pool", bufs=1) as bias_pool,
                tc.tile_pool(name="stage", bufs=2) as stage_pool,
                tc.tile_pool(name="ppsum", bufs=4, space="PSUM") as ppsum,
            ):
                hs_sb = hs_pool.tile([P, NK, S], f32r)
                for c in range(8):  # split the 16.8MB load across DMA queues
                    nc.gpsimd.dma_start(
                        hs_sb[:, 2 * c : 2 * c + 2, :], hsT_r[:, 2 * c : 2 * c + 2, :]
                    )
                bias_sb = bias_pool.tile([P, 3, NM], f32)
                nc.sync.dma_start(
                    bias_sb, bqkv.rearrange("w (mo p) -> p w mo", p=P)
                )
                bv_rep = bias_pool.tile([P, ES], f32)
                nc.sync.dma_start(
                    bv_rep, bqkv[2:3, :].to_broadcast([P, ES])
                )
                ones8 = bias_pool.tile([P, HG], f32)
                nc.any.memset(ones8, 1.0)

                # q and k: out tiles [e' 128, s 512]
                for w_idx, w_ap, out_d in ((0, wqT, qT_d), (1, wkT, kT_d)):
                    w_r = w_ap.rearrange("(ko p) m -> p ko m", p=P)
                    for m in range(NM):
                        w_sb = wp.tile([P, NK, P], f32r, tag="wqk")
                        nc.gpsimd.dma_start(w_sb, w_r[:, :, m * P : (m + 1) * P])
                        for sb in range(NSB):
                            ps = ppsum.tile([P, FD], f32, tag="proj")
                            for k in range(NK):
                                _mm(
                                    nc, ps,
                                    lhsT=w_sb[:, k, :],
                                    rhs=hs_sb[:, k, sb * FD : (sb + 1) * FD],
                                    start=(k == 0),
                                    stop=(k == NK - 1),
                                )
                            st = stage_pool.tile([P, FD], f32r, tag="qk_st")
                            nc.vector.tensor_scalar(
                                out=st, in0=ps,
                                scalar1=bias_sb[:, w_idx, m : m + 1], scalar2=None,
                                op0=ALU.add,
                            )
                            nc.sync.dma_start(
                                out_d[m * P : (m + 1) * P, sb * FD : (sb + 1) * FD], st
                            )

                # v: out tiles [s 128, e' 512], interleaved with ones columns
                wv_r = wvT.rearrange("(ko p) m -> p ko m", p=P)
                wv_sb = wp.tile([P, NK, ES], f32r, tag="wv", bufs=1)
                for c in range(4):
                    nc.gpsimd.dma_start(
                        wv_sb[:, 4 * c : 4 * c + 4, :], wv_r[:, 4 * c : 4 * c + 4, :]
                    )
                for so in range(NT):
                    ps = ppsum.tile([P, ES], f32, tag="proj")
                    for k in range(NK):
                        _mm(
                            nc, ps,
                            lhsT=hs_sb[:, k, so * P : (so + 1) * P],
                            rhs=wv_sb[:, k, :],
                            start=(k == 0),
                            stop=(k == NK - 1),
                        )
                    vst = stage_pool.tile([P, HG * CW], f32r, tag="v_st")
                    for h in range(HG):
                        nc.any.tensor_add(
                            out=vst[:, h * CW : h * CW + D],
                            in0=ps[:, h * D : (h + 1) * D],
                            in1=bv_rep[:, h * D : (h + 1) * D],
                        )
                    nc.any.tensor_copy(out=vst[:, D :: CW], in_=ones8)
                    nc.sync.dma_start(v_d[so * P : (so + 1) * P, :], vst)

            # ---------------- phase 2: attention ----------------
            with (
                tc.tile_pool(name="qkv_res", bufs=1) as qkv_res,
                tc.tile_pool(name="mask_pool", bufs=1) as mask_pool,
                tc.tile_pool(name="p_pool", bufs=2) as p_pool,
                tc.tile_pool(name="cstage", bufs=3) as cstage,
                tc.tile_pool(name="sc_psum", bufs=4, space="PSUM") as sc_psum,
                tc.tile_pool(name="pv_psum", bufs=2, space="PSUM") as pv_psum,
            ):
                qT_sb = qkv_res.tile([P, NM, S], f32r)
                kT_sb = qkv_res.tile([P, NM, S], f32r)
                v_sb = qkv_res.tile([P, NT, HG * CW], f32r)
                qr = qT_d.rearrange("(mo p) s -> p mo s", p=P)
                kr = kT_d.rearrange("(mo p) s -> p mo s", p=P)
                vr = v_d.rearrange("(so p) c -> p so c", p=P)
                for c in range(NM):
                    nc.sync.dma_start(qT_sb[:, c : c + 1, :], qr[:, c : c + 1, :])
                    nc.sync.dma_start(kT_sb[:, c : c + 1, :], kr[:, c : c + 1, :])
                for c in range(4):
                    nc.sync.dma_start(
                        v_sb[:, 4 * c : 4 * c + 4, :], vr[:, 4 * c : 4 * c + 4, :]
                    )

                for sb in range(NSB):
                    mk_sb = mask_pool.tile([P, NT, FD], f32, tag="mk")
                    for c in range(4):
                        nc.sync.dma_start(
                            mk_sb[:, 4 * c : 4 * c + 4, :],
                            maskT_r[:, 4 * c : 4 * c + 4, sb * FD : (sb + 1) * FD],
                        )
                    for j in range(HG // 2):  # head pairs
                        pv_ps = [
                            pv_psum.tile([CW, FD], f32, name=f"pv{i}", tag=f"pv{i}")
                            for i in range(2)
                        ]
                        for tb in range(NT // EXPB):  # exp batches
                            pt = [
                                p_pool.tile([P, EXPB, FD], f32r, name=f"pt{i}",
                                            tag=f"pt{i}")
                                for i in range(2)
                            ]
                            for te in range(EXPB):
                                t = tb * EXPB + te
                                for i in range(2):
                                    h = 2 * j + i
                                    lo = D * (h % 2)
                                    sc = sc_psum.tile([P, FD], f32, tag="sc")
                                    _mm(
                                        nc, sc,
                                        lhsT=kT_sb[lo : lo + D, j, t * P : (t + 1) * P],
                                        rhs=qT_sb[lo : lo + D, j,
                                                  sb * FD : (sb + 1) * FD],
                                        start=True,
                                        stop=True,
                                    )
                                    nc.vector.tensor_tensor(
                                        pt[i][:, te, :], sc, mk_sb[:, t, :], ALU.add
                                    )
                            for i in range(2):
                                nc.scalar.activation(pt[i], pt[i], AF.Exp)
                            for te in range(EXPB):
                                t = tb * EXPB + te
                                for i in range(2):
                                    h = 2 * j + i
                                    _mm(
                                        nc, pv_ps[i],
                                        lhsT=v_sb[:, t, h * CW : (h + 1) * CW],
                                        rhs=pt[i][:, te, :],
                                        start=(t == 0),
                                        stop=(t == NT - 1),
                                    )
                        for i in range(2):
                            h = 2 * j + i
                            cst = cstage.tile([CW, FD], f32, tag="cst")
                            nc.any.tensor_copy(out=cst, in_=pv_ps[i])
                            nc.sync.dma_start(
                                ctxT[h * CW : (h + 1) * CW, sb * FD : (sb + 1) * FD],
                                cst,
                            )
    nc.compile()
    return nc


def build_launch2():
    RPC = B * S // 8  # 512 rows per core
    nc = bacc.Bacc(None, target_bir_lowering=False)
    xc = nc.declare_dram_parameter("xc", [RPC, E], f32, isOutput=False)
    w2 = nc.declare_dram_parameter("w2", [E, E], f32, isOutput=False)
    b2 = nc.declare_dram_parameter("b2", [1, E], f32, isOutput=False)
    outr = nc.declare_dram_parameter("outr", [RPC, E], f32, isOutput=True)

    NMT = RPC // P  # 4 row tiles
    NNT = E // FD   # 4 out-column tiles
    w2_r = w2.rearrange("(ko p) e -> p ko e", p=P)
    xc_r = xc.rearrange("(mo p) e -> p mo e", p=P)

    with tile.TileContext(nc) as tc:
        with (
            tc.tile_pool(name="const2", bufs=1) as const2,
            tc.tile_pool(name="xp", bufs=2) as xp,
            tc.tile_pool(name="statp", bufs=4) as statp,
            tc.tile_pool(name="ytp", bufs=1) as ytp,
            tc.tile_pool(name="w2p", bufs=2) as w2p,
            tc.tile_pool(name="ostage", bufs=3) as ostage,
            tc.tile_pool(name="tpsum", bufs=2, space="PSUM") as tpsum,
            tc.tile_pool(name="opsum", bufs=3, space="PSUM") as opsum,
        ):
            ident = const2.tile([P, P], f32)
            make_identity(nc, ident)
            b2_rep = const2.tile([P, E], f32)
            nc.sync.dma_start(b2_rep, b2[0:1, :].to_broadcast([P, E]))
            eps_sb = const2.tile([P, 1], f32)
            nc.any.memset(eps_sb, LN_EPS)
            yT = ytp.tile([P, NK, RPC], f32r)

            for mt in range(NMT):
                x = xp.tile([P, E], f32, tag="x")
                nc.sync.dma_start(x, xc_r[:, mt, :])
                sq = xp.tile([P, E], f32, tag="sq")
                nc.scalar.activation(sq, x, AF.Square)
                s1 = statp.tile([P, 1], f32, tag="s1")
                s2 = statp.tile([P, 1], f32, tag="s2")
                nc.vector.reduce_sum(s1, x, axis=mybir.AxisListType.X)
                nc.vector.reduce_sum(s2, sq, axis=mybir.AxisListType.X)
                mu = statp.tile([P, 1], f32, tag="mu")
                nc.vector.tensor_scalar_mul(mu, s1, 1.0 / E)
                var = statp.tile([P, 1], f32, tag="var")
                # var = s2/E - mu^2
                musq = statp.tile([P, 1], f32, tag="musq")
                nc.vector.tensor_tensor(musq, mu, mu, ALU.mult)
                nc.vector.tensor_scalar(
                    out=var, in0=s2, scalar1=1.0 / E, scalar2=None, op0=ALU.mult
                )
                nc.vector.tensor_tensor(var, var, musq, ALU.subtract)
                sd = statp.tile([P, 1], f32, tag="sd")
                nc.scalar.activation(sd, var, AF.Sqrt, bias=eps_sb)
                r = statp.tile([P, 1], f32, tag="r")
                nc.vector.reciprocal(r, sd)
                nmr = statp.tile([P, 1], f32, tag="nmr")
                nc.vector.tensor_tensor(nmr, mu, r, ALU.mult)
                nc.vector.tensor_scalar_mul(nmr, nmr, -1.0)
                y = xp.tile([P, E], f32, tag="y")
                nc.vector.tensor_scalar(
                    out=y, in0=x, scalar1=r, scalar2=nmr, op0=ALU.mult, op1=ALU.add
                )
                for k in range(NK):
                    tp = tpsum.tile([P, P], f32, tag="tp")
                    nc.tensor.transpose(tp, y[:, k * P : (k + 1) * P], ident)
                    nc.any.tensor_copy(
                        out=yT[:, k, mt * P : (mt + 1) * P], in_=tp
                    )

            for nt in range(NNT):
                w_sb = w2p.tile([P, NK, FD], f32r, tag="w2")
                for c in range(4):
                    nc.gpsimd.dma_start(
                        w_sb[:, 4 * c : 4 * c + 4, :],
                        w2_r[:, 4 * c : 4 * c + 4, nt * FD : (nt + 1) * FD],
                    )
                for mt in range(NMT):
                    ps = opsum.tile([P, FD], f32, tag="ops")
                    for k in range(NK):
                        _mm(
                            nc, ps,
                            lhsT=yT[:, k, mt * P : (mt + 1) * P],
                            rhs=w_sb[:, k, :],
                            start=(k == 0),
                            stop=(k == NK - 1),
                        )
                    ost = ostage.tile([P, FD], f32, tag="ost")
                    nc.any.tensor_add(
                        out=ost, in0=ps, in1=b2_rep[:, nt * FD : (nt + 1) * FD]
                    )
                    nc.sync.dma_start(
                        outr.rearrange("(mo p) e -> p mo e", p=P)[
                            :, mt, nt * FD : (nt + 1) * FD
                        ],
                        ost,
                    )
    nc.compile()
    return nc


def _prep_launch1_inputs(hidden_states, attention_mask, Wq, bq, Wk, bk, Wv, bv):
    hsT = [np.ascontiguousarray(hidden_states[b].T) for b in range(B)]
    maskT = [np.ascontiguousarray(attention_mask[b, 0].T) for b in range(B)]
    in_maps = []
    for c in range(8):
        b, g = c // G, c % G
        sl = slice(g * ES, (g + 1) * ES)
        in_maps.append({
            "hsT": hsT[b],
            "wqT": np.ascontiguousarray(Wq[sl, :].T * SCALE),
            "wkT": np.ascontiguousarray(Wk[sl, :].T),
            "wvT": np.ascontiguousarray(Wv[sl, :].T),
            "bqkv": np.ascontiguousarray(
                np.stack([bq[sl] * SCALE, bk[sl], bv[sl]])
            ),
            "maskT": maskT[b],
        })
    return in_maps


def _assemble_ctx(results1):
    """results1[c]["ctxT"] [520, 2048] -> full ctx [B*S, E]."""
    ctx = np.empty((B * S, E), dtype=np.float32)
    for c in range(8):
        b, g = c // G, c % G
        arr = results1[c]["ctxT"].reshape(HG, CW, S)
        normed = arr[:, :D, :] / arr[:, D : D + 1, :]   # [HG, D, S]
        # -> [S, HG*D]
        ctx[b * S : (b + 1) * S, g * ES : (g + 1) * ES] = (
            normed.transpose(2, 0, 1).reshape(S, ES)
        )
    return ctx


def run_pipeline(inputs, trace=False):
    hidden_states = np.asarray(inputs["hidden_states"], dtype=np.float32)
    attention_mask = np.asarray(inputs["attention_mask"], dtype=np.float32)
    Wq = np.asarray(inputs["Wq"], dtype=np.float32)
    Wk = np.asarray(inputs["Wk"], dtype=np.float32)
    Wv = np.asarray(inputs["Wv"], dtype=np.float32)
    Wo = np.asarray(inputs["Wo"], dtype=np.float32)
    bq = np.asarray(inputs["bq"], dtype=np.float32)
    bk = np.asarray(inputs["bk"], dtype=np.float32)
    bv = np.asarray(inputs["bv"], dtype=np.float32)
    bo = np.asarray(inputs["bo"], dtype=np.float32)
    ln_gamma = np.asarray(inputs["ln_gamma"], dtype=np.float32)
    ln_beta = np.asarray(inputs["ln_beta"], dtype=np.float32)

    core_ids = list(range(8))
    nc1 = build_launch1()
    in_maps1 = _prep_launch1_inputs(
        hidden_states, attention_mask, Wq, bq, Wk, bk, Wv, bv
    )
    res1 = run_bass_kernel_spmd(nc1, in_maps1, core_ids, trace=trace)
    ctx = _assemble_ctx(res1.results)

    # launch 2: fold gamma/beta into out-proj
    w2 = np.ascontiguousarray(ln_gamma[:, None] * Wo.T)
    b2 = np.ascontiguousarray((ln_beta @ Wo.T + bo)[None, :])
    RPC = B * S // 8
    nc2 = build_launch2()
    in_maps2 = [
        {"xc": np.ascontiguousarray(ctx[c * RPC : (c + 1) * RPC]), "w2": w2,
         "b2": b2}
        for c in range(8)
    ]
    res2 = run_bass_kernel_spmd(nc2, in_maps2, core_ids, trace=trace)
    out = np.concatenate([res2.results[c]["outr"] for c in range(8)], axis=0)
    out = out.reshape(B, S, E)
    ns = None
    if trace:
        parts = [r.exec_time_ns for r in (res1, res2)]
        if all(p is not None for p in parts):
            ns = sum(parts)
    return out, ns, (res1, res2)


def kernel(**inputs):
    out, _, _ = run_pipeline(inputs, trace=False)
    return out
